# revision 11
# baseline (speedup 1.0000x reference)
"""CRF NLL (forward-algorithm partition function) on 8 Trainium2 NeuronCores.

Math: the reference computes  mean_b( logZ[b] - score[b] )  where
  logZ = logsumexp forward recursion over S=2048 steps with transition
  matrix T [L,L], emissions [B,S,L], and score is a pure gather path.

Device strategy (pure data parallel, batch sharded 8 ways, 16 seq/core):
  Linear-space recursion in layout [l (partitions), b (free)]:
      p_{t+1} = (expT^T @ p_t) * exp(em[:,t,:] - D)^T
  - PE matmul with stationary expT (lhsT, natural layout), rhs = p.
  - One DVE tensor_mul per step fuses the emission factor (PSUM -> SBUF).
  - D ~ mean log-growth per step keeps p O(1); every K_RESCALE steps an
    off-critical-path colsum (PE ones-matmul) + reciprocal (DVE) +
    broadcast (PE ones-matmul) rescales p exactly, with log(s) accumulated
    into c[b]; the scale is folded into a future emission tile so the
    critical chain stays exactly matmul -> tensor_mul per step.
  - Final: s_fin = expEnd @ p_S (PE), logZ_dev = ln(s_fin) + c (ACT+DVE).
Host: exp/transpose of emissions (pre), score gathers + mean (post).
"""

import os
from contextlib import ExitStack

import numpy as np

B, S, L = 128, 2048, 128
NCORES = 8
BS = B // NCORES  # 16 sequences per core
IGNORE = -100

D_SHIFT = 5.829        # expected per-step log growth (measured offline)
K_RESCALE = 64         # exact rescale cadence (steps)
APPLY_DELTA = 4        # steps between measuring colsum and applying 1/s
T_CHUNK = 256          # emission steps per DMA chunk

USE_BF16 = True        # compute dtype for PE inputs / p / emission tiles
                       # (fp32 PSUM accumulation; HW-measured 268 ns/step vs
                       # 854 ns/step for fp32 inputs; logZ rel err ~1e-5)

# test.py introspection
LAST_EXEC_TIME_NS = None
LAST_TRACE_PATH = None

_BUILT = {}


def _build(nsteps, repeat=1, use_bf16=False):
    """Build the Bass/Tile program for `nsteps` recursion steps (S-1 real).

    repeat > 1 wraps the whole recursion in a hardware For loop that re-runs
    it `repeat` times on identical inputs — used only by dev timing to lift
    the ~1 ms kernel above the ~60 ms axon dispatch noise floor.

    use_bf16 stores expT / p / emission tiles in bf16 (PE fast-weight-load,
    fp32 PSUM accumulation kept). Validated rel err ~7e-6 on logZ.
    """
    import concourse.bacc as bacc
    import concourse.tile as tile
    from concourse import mybir

    f32 = mybir.dt.float32
    cdt = mybir.dt.bfloat16 if use_bf16 else f32
    Ln = mybir.ActivationFunctionType.Ln

    nc = bacc.Bacc(debug=False, name="crf_fwd")
    with tile.TileContext(nc) as tc:
        with ExitStack() as ctx:
            d_expT = nc.dram_tensor("expT", [L, L], cdt, kind="ExternalInput")
            d_expEnd = nc.dram_tensor("expEnd", [L, 1], cdt, kind="ExternalInput")
            d_p0 = nc.dram_tensor("p0", [L, BS], cdt, kind="ExternalInput")
            d_E = nc.dram_tensor("emis", [L, nsteps, BS], cdt, kind="ExternalInput")
            d_out = nc.dram_tensor("out", [1, BS], f32, kind="ExternalOutput")

            const = ctx.enter_context(tc.tile_pool(name="const", bufs=1))
            empool = ctx.enter_context(tc.tile_pool(name="empool", bufs=3))
            ppool = ctx.enter_context(tc.tile_pool(name="ppool", bufs=4))
            sclp = ctx.enter_context(tc.tile_pool(name="sclp", bufs=2))
            smalls = ctx.enter_context(tc.tile_pool(name="smalls", bufs=4))
            zpsum = ctx.enter_context(tc.tile_pool(name="zpsum", bufs=3, space="PSUM"))
            spsum = ctx.enter_context(tc.tile_pool(name="spsum", bufs=2, space="PSUM"))
            rpsum = ctx.enter_context(tc.tile_pool(name="rpsum", bufs=2, space="PSUM"))

            expT_sb = const.tile([L, L], cdt)
            nc.sync.dma_start(out=expT_sb, in_=d_expT[:])
            expEnd_sb = const.tile([L, 1], cdt)
            nc.sync.dma_start(out=expEnd_sb, in_=d_expEnd[:])
            ones_col = const.tile([L, 1], cdt)
            nc.vector.memset(ones_col, 1.0)
            ones_row = const.tile([1, L], f32)
            nc.vector.memset(ones_row, 1.0)
            c_sb = const.tile([1, BS], f32)
            nc.vector.memset(c_sb, 0.0)

            def _run_chain():
                p_cur = ppool.tile([L, BS], cdt, tag="p")
                nc.sync.dma_start(out=p_cur, in_=d_p0[:])

                # apply-step -> PSUM broadcast tile of 1/s
                pending = {}
                em_tile = None
                chunk_lo = -1

                for t in range(1, nsteps + 1):
                    i = t - 1  # emission index in d_E
                    if i // T_CHUNK != chunk_lo:
                        chunk_lo = i // T_CHUNK
                        lo = chunk_lo * T_CHUNK
                        hi = min(lo + T_CHUNK, nsteps)
                        em_tile = empool.tile([L, T_CHUNK, BS], cdt, tag="em")
                        nc.sync.dma_start(
                            out=em_tile[:, : hi - lo, :], in_=d_E[:, lo:hi, :]
                        )
                    em_sl = em_tile[:, i % T_CHUNK, :]

                    if t in pending:
                        rbc = pending.pop(t)
                        em_scaled = sclp.tile([L, BS], cdt, tag="scl")
                        nc.vector.tensor_mul(em_scaled, em_sl, rbc)
                        em_sl = em_scaled

                    z = zpsum.tile([L, BS], f32, tag="z")
                    nc.tensor.matmul(
                        z, lhsT=expT_sb, rhs=p_cur, start=True, stop=True
                    )
                    p_new = ppool.tile([L, BS], cdt, tag="p")
                    nc.vector.tensor_mul(p_new, z, em_sl)
                    p_cur = p_new

                    if t % K_RESCALE == 0 and t + APPLY_DELTA <= nsteps:
                        s_ps = spsum.tile([1, BS], f32, tag="s")
                        nc.tensor.matmul(
                            s_ps, lhsT=ones_col, rhs=p_cur, start=True, stop=True
                        )
                        r_sb = smalls.tile([1, BS], f32, tag="r")
                        nc.vector.reciprocal(r_sb, s_ps)
                        rbc = rpsum.tile([L, BS], f32, tag="rbc")
                        nc.tensor.matmul(
                            rbc, lhsT=ones_row, rhs=r_sb, start=True, stop=True
                        )
                        ln_s = smalls.tile([1, BS], f32, tag="lns")
                        nc.scalar.activation(ln_s, s_ps, Ln)
                        nc.vector.tensor_add(c_sb, c_sb, ln_s)
                        pending[t + APPLY_DELTA] = rbc

                s_fin = spsum.tile([1, BS], f32, tag="s")
                nc.tensor.matmul(
                    s_fin, lhsT=expEnd_sb, rhs=p_cur, start=True, stop=True
                )
                ln_fin = smalls.tile([1, BS], f32, tag="lns")
                nc.scalar.activation(ln_fin, s_fin, Ln)
                outv = smalls.tile([1, BS], f32, tag="outv")
                nc.vector.tensor_add(outv, ln_fin, c_sb)
                nc.sync.dma_start(out=d_out[:], in_=outv)

            if repeat == 1:
                _run_chain()
            else:
                with tc.For_i(0, repeat, 1):
                    _run_chain()

    nc.compile()
    return nc


def _get_program(nsteps, repeat=1, use_bf16=None):
    if use_bf16 is None:
        use_bf16 = USE_BF16
    key = (nsteps, repeat, use_bf16)
    if key not in _BUILT:
        _BUILT[key] = _build(nsteps, repeat, use_bf16)
    return _BUILT[key]


def _prepare_in_maps(emissions, transitions, start_transitions, end_transitions,
                     nsteps=S - 1, use_bf16=None):
    """Host preprocessing -> (in_maps for 8 cores, c0[B])."""
    if use_bf16 is None:
        use_bf16 = USE_BF16
    if use_bf16:
        import ml_dtypes
        cdt = ml_dtypes.bfloat16
    else:
        cdt = np.float32
    expT = np.exp(transitions, dtype=np.float32).astype(cdt)      # [l, l']
    expEnd = np.exp(end_transitions, dtype=np.float32).reshape(L, 1).astype(cdt)

    alpha0 = start_transitions[None, :] + emissions[:, 0, :]      # [B, L] f32
    c0 = alpha0.max(axis=1)                                        # [B]
    p0_all = np.exp(alpha0 - c0[:, None]).T.astype(np.float32)     # [l, B]

    in_maps = []
    for k in range(NCORES):
        bs = slice(k * BS, (k + 1) * BS)
        # [l, t, b] = exp(em - D) transposed; steps 1..nsteps
        Ek = np.exp(
            np.ascontiguousarray(
                np.transpose(emissions[bs, 1 : nsteps + 1, :], (2, 1, 0))
            )
            - np.float32(D_SHIFT),
            dtype=np.float32,
        ).astype(cdt)
        in_maps.append(
            {
                "expT": expT,
                "expEnd": expEnd,
                "p0": np.ascontiguousarray(p0_all[:, bs]).astype(cdt),
                "emis": Ek,
            }
        )
    return in_maps, c0


def _forward_device(emissions, transitions, start_transitions, end_transitions,
                    nsteps=S - 1):
    """Run the device recursion; returns logZ [B] float64."""
    from concourse.bass_utils import run_bass_kernel_spmd

    global LAST_EXEC_TIME_NS, LAST_TRACE_PATH

    in_maps, c0 = _prepare_in_maps(
        emissions, transitions, start_transitions, end_transitions, nsteps
    )
    nc = _get_program(nsteps)
    trace = os.environ.get("CRF_TRACE", "") == "1"
    res = run_bass_kernel_spmd(
        nc, in_maps, core_ids=list(range(NCORES)), trace=trace
    )
    LAST_EXEC_TIME_NS = res.exec_time_ns
    if res.instructions_and_trace is not None:
        LAST_TRACE_PATH = res.instructions_and_trace[1]

    out = np.concatenate([res.results[k]["out"][0] for k in range(NCORES)])
    return out.astype(np.float64) + c0.astype(np.float64) + D_SHIFT * nsteps


def _score_host(emissions, mask, tags, transitions, start_transitions,
                end_transitions):
    """Gold path score, matching reference._crf_nll's gather path. float64."""
    em = emissions.astype(np.float64)
    T = transitions.astype(np.float64)
    startT = start_transitions.astype(np.float64)
    endT = end_transitions.astype(np.float64)

    valid = tags != IGNORE
    tags_safe = np.where(valid, tags, 0).astype(np.int64)
    vf = valid.astype(np.float64)

    score = startT[tags_safe[:, 0]] * vf[:, 0]
    prev_t = tags_safe[:, :-1]
    curr_t = tags_safe[:, 1:]
    trans_sc = T[prev_t, curr_t]
    em_sc = np.take_along_axis(em[:, 1:, :], curr_t[:, :, None], axis=2)[..., 0]
    score = score + np.sum((trans_sc + em_sc) * vf[:, 1:], axis=1)

    pos = np.arange(tags.shape[1])
    last_idx = np.max(np.where(valid, pos[None, :], -1), axis=1)
    last_tag = tags_safe[np.arange(tags.shape[0]), np.clip(last_idx, 0, S - 1)]
    score = score + np.where(last_idx >= 0, endT[last_tag], 0.0)
    return score


def _forward_numpy(emissions, mask, transitions, start_transitions,
                   end_transitions):
    """Fallback exact forward recursion (used only if mask isn't all ones)."""
    em = emissions.astype(np.float64)
    T = transitions.astype(np.float64)
    alpha = start_transitions.astype(np.float64)[None, :] + em[:, 0, :]
    for t in range(1, em.shape[1]):
        m = alpha.max(axis=1, keepdims=True)
        new = m + np.log(np.exp(alpha - m) @ np.exp(T)) + em[:, t, :]
        alpha = np.where(mask[:, t][:, None], new, alpha)
    m = alpha.max(axis=1, keepdims=True)
    return (
        m[:, 0]
        + np.log(
            np.exp(alpha - m) @ np.exp(end_transitions.astype(np.float64))
        )
    )


def kernel(emissions, mask, tags, transitions, start_transitions,
           end_transitions):
    emissions = np.asarray(emissions, dtype=np.float32)
    mask = np.asarray(mask)
    tags = np.asarray(tags)
    transitions = np.asarray(transitions, dtype=np.float32)
    start_transitions = np.asarray(start_transitions, dtype=np.float32)
    end_transitions = np.asarray(end_transitions, dtype=np.float32)

    if bool(mask.all()):
        logz = _forward_device(
            emissions, transitions, start_transitions, end_transitions
        )
    else:
        logz = _forward_numpy(
            emissions, mask, transitions, start_transitions, end_transitions
        )

    score = _score_host(
        emissions, mask, tags, transitions, start_transitions, end_transitions
    )
    return np.asarray(np.mean(logz - score), dtype=np.float32)


# revision 16
# speedup vs baseline: 131.5525x; 131.5525x over previous
"""CRF NLL (forward-algorithm partition function) on 8 Trainium2 NeuronCores.

Math: the reference computes  mean_b( logZ[b] - score[b] )  where
  logZ = logsumexp forward recursion over S=2048 steps with transition
  matrix T [L,L], emissions [B,S,L], and score is a pure gather path.

Device strategy (pure data parallel, batch sharded 8 ways, 16 seq/core):
  Linear-space recursion in layout [l (partitions), b (free)]:
      p_{t+1} = (expT^T @ p_t) * exp(em[:,t,:] - D)^T
  - PE matmul with stationary expT (lhsT, natural layout), rhs = p.
  - One DVE tensor_mul per step fuses the emission factor (PSUM -> SBUF).
  - D ~ mean log-growth per step keeps p O(1); every K_RESCALE steps an
    off-critical-path colsum (PE ones-matmul) + reciprocal (DVE) +
    broadcast (PE ones-matmul) rescales p exactly, with log(s) accumulated
    into c[b]; the scale is folded into a future emission tile so the
    critical chain stays exactly matmul -> tensor_mul per step.
  - Final: s_fin = expEnd @ p_S (PE), logZ_dev = ln(s_fin) + c (ACT+DVE).
Host: exp/transpose of emissions (pre), score gathers + mean (post).

Shipped config: _build_fb2 — forward-backward split (alpha forward to the
midpoint, beta backward from the end, combined as logZ = log sum_l pA*pB),
both chains' emission multiplies fused into one [L, 2*BS] DVE op per slot.
HW-measured ~239 us/kernel (~233 ns/slot) vs 1.75 ms for the naive fp32
single-direction chain.
"""

import os
from contextlib import ExitStack

import numpy as np

B, S, L = 128, 2048, 128
NCORES = 8
BS = B // NCORES  # 16 sequences per core
IGNORE = -100

D_SHIFT = 5.829        # expected per-step log growth (measured offline)
K_RESCALE = 64         # exact rescale cadence (steps)
APPLY_DELTA = 4        # steps between measuring colsum and applying 1/s
T_CHUNK = 256          # emission steps per DMA chunk

USE_BF16 = True        # compute dtype for PE inputs / p / emission tiles
                       # (fp32 PSUM accumulation; HW-measured 268 ns/step vs
                       # 854 ns/step for fp32 inputs; logZ rel err ~1e-5)

# test.py introspection
LAST_EXEC_TIME_NS = None
LAST_TRACE_PATH = None

_BUILT = {}


def _build(nsteps, repeat=1, use_bf16=False):
    """Build the Bass/Tile program for `nsteps` recursion steps (S-1 real).

    repeat > 1 wraps the whole recursion in a hardware For loop that re-runs
    it `repeat` times on identical inputs — used only by dev timing to lift
    the ~1 ms kernel above the ~60 ms axon dispatch noise floor.

    use_bf16 stores expT / p / emission tiles in bf16 (PE fast-weight-load,
    fp32 PSUM accumulation kept). Validated rel err ~7e-6 on logZ.
    """
    import concourse.bacc as bacc
    import concourse.tile as tile
    from concourse import mybir

    f32 = mybir.dt.float32
    cdt = mybir.dt.bfloat16 if use_bf16 else f32
    Ln = mybir.ActivationFunctionType.Ln

    nc = bacc.Bacc(debug=False, name="crf_fwd")
    with tile.TileContext(nc) as tc:
        with ExitStack() as ctx:
            d_expT = nc.dram_tensor("expT", [L, L], cdt, kind="ExternalInput")
            d_expEnd = nc.dram_tensor("expEnd", [L, 1], cdt, kind="ExternalInput")
            d_p0 = nc.dram_tensor("p0", [L, BS], cdt, kind="ExternalInput")
            d_E = nc.dram_tensor("emis", [L, nsteps, BS], cdt, kind="ExternalInput")
            d_out = nc.dram_tensor("out", [1, BS], f32, kind="ExternalOutput")

            const = ctx.enter_context(tc.tile_pool(name="const", bufs=1))
            empool = ctx.enter_context(tc.tile_pool(name="empool", bufs=3))
            ppool = ctx.enter_context(tc.tile_pool(name="ppool", bufs=4))
            sclp = ctx.enter_context(tc.tile_pool(name="sclp", bufs=2))
            smalls = ctx.enter_context(tc.tile_pool(name="smalls", bufs=4))
            zpsum = ctx.enter_context(tc.tile_pool(name="zpsum", bufs=3, space="PSUM"))
            spsum = ctx.enter_context(tc.tile_pool(name="spsum", bufs=2, space="PSUM"))
            rpsum = ctx.enter_context(tc.tile_pool(name="rpsum", bufs=2, space="PSUM"))

            expT_sb = const.tile([L, L], cdt)
            nc.sync.dma_start(out=expT_sb, in_=d_expT[:])
            expEnd_sb = const.tile([L, 1], cdt)
            nc.sync.dma_start(out=expEnd_sb, in_=d_expEnd[:])
            ones_col = const.tile([L, 1], cdt)
            nc.vector.memset(ones_col, 1.0)
            ones_row = const.tile([1, L], f32)
            nc.vector.memset(ones_row, 1.0)
            c_sb = const.tile([1, BS], f32)
            nc.vector.memset(c_sb, 0.0)

            def _run_chain():
                p_cur = ppool.tile([L, BS], cdt, tag="p")
                nc.sync.dma_start(out=p_cur, in_=d_p0[:])

                # apply-step -> PSUM broadcast tile of 1/s
                pending = {}
                em_tile = None
                chunk_lo = -1

                for t in range(1, nsteps + 1):
                    i = t - 1  # emission index in d_E
                    if i // T_CHUNK != chunk_lo:
                        chunk_lo = i // T_CHUNK
                        lo = chunk_lo * T_CHUNK
                        hi = min(lo + T_CHUNK, nsteps)
                        em_tile = empool.tile([L, T_CHUNK, BS], cdt, tag="em")
                        nc.sync.dma_start(
                            out=em_tile[:, : hi - lo, :], in_=d_E[:, lo:hi, :]
                        )
                    em_sl = em_tile[:, i % T_CHUNK, :]

                    if t in pending:
                        rbc = pending.pop(t)
                        em_scaled = sclp.tile([L, BS], cdt, tag="scl")
                        nc.vector.tensor_mul(em_scaled, em_sl, rbc)
                        em_sl = em_scaled

                    z = zpsum.tile([L, BS], f32, tag="z")
                    nc.tensor.matmul(
                        z, lhsT=expT_sb, rhs=p_cur, start=True, stop=True
                    )
                    p_new = ppool.tile([L, BS], cdt, tag="p")
                    nc.vector.tensor_mul(p_new, z, em_sl)
                    p_cur = p_new

                    if t % K_RESCALE == 0 and t + APPLY_DELTA <= nsteps:
                        s_ps = spsum.tile([1, BS], f32, tag="s")
                        nc.tensor.matmul(
                            s_ps, lhsT=ones_col, rhs=p_cur, start=True, stop=True
                        )
                        r_sb = smalls.tile([1, BS], f32, tag="r")
                        nc.vector.reciprocal(r_sb, s_ps)
                        rbc = rpsum.tile([L, BS], f32, tag="rbc")
                        nc.tensor.matmul(
                            rbc, lhsT=ones_row, rhs=r_sb, start=True, stop=True
                        )
                        ln_s = smalls.tile([1, BS], f32, tag="lns")
                        nc.scalar.activation(ln_s, s_ps, Ln)
                        nc.vector.tensor_add(c_sb, c_sb, ln_s)
                        pending[t + APPLY_DELTA] = rbc

                s_fin = spsum.tile([1, BS], f32, tag="s")
                nc.tensor.matmul(
                    s_fin, lhsT=expEnd_sb, rhs=p_cur, start=True, stop=True
                )
                ln_fin = smalls.tile([1, BS], f32, tag="lns")
                nc.scalar.activation(ln_fin, s_fin, Ln)
                outv = smalls.tile([1, BS], f32, tag="outv")
                nc.vector.tensor_add(outv, ln_fin, c_sb)
                nc.sync.dma_start(out=d_out[:], in_=outv)

            if repeat == 1:
                _run_chain()
            else:
                with tc.For_i(0, repeat, 1):
                    _run_chain()

    nc.compile()
    return nc




def _build_fb(nsteps, repeat=1):
    """Forward-backward split builder (bf16): alpha runs t=1..F forward,
    beta runs t=nsteps..F+1 backward, both chains interleaved per slot;
    logZ = log(sum_l pA*pB) + corrections. Halves the serial chain length."""
    import concourse.bacc as bacc
    import concourse.tile as tile
    from concourse import mybir

    f32 = mybir.dt.float32
    cdt = mybir.dt.bfloat16
    Ln = mybir.ActivationFunctionType.Ln
    F = nsteps // 2          # forward chain steps (t = 1..F)
    G = nsteps - F           # backward chain steps (t = nsteps..F+1)

    nc = bacc.Bacc(debug=False, name="crf_fb")
    with tile.TileContext(nc) as tc:
        with ExitStack() as ctx:
            d_expT = nc.dram_tensor("expT", [L, L], cdt, kind="ExternalInput")
            d_expTT = nc.dram_tensor("expTT", [L, L], cdt, kind="ExternalInput")
            d_p0 = nc.dram_tensor("p0", [L, BS], cdt, kind="ExternalInput")
            d_b0 = nc.dram_tensor("b0", [L, BS], cdt, kind="ExternalInput")
            d_E = nc.dram_tensor("emis", [L, nsteps, BS], cdt, kind="ExternalInput")
            d_out = nc.dram_tensor("out", [1, BS], f32, kind="ExternalOutput")

            const = ctx.enter_context(tc.tile_pool(name="const", bufs=1))
            empool = ctx.enter_context(tc.tile_pool(name="empool", bufs=3))
            pApool = ctx.enter_context(tc.tile_pool(name="pApool", bufs=4))
            pBpool = ctx.enter_context(tc.tile_pool(name="pBpool", bufs=4))
            sclp = ctx.enter_context(tc.tile_pool(name="sclp", bufs=2))
            smalls = ctx.enter_context(tc.tile_pool(name="smalls", bufs=4))
            zApsum = ctx.enter_context(tc.tile_pool(name="zApsum", bufs=2, space="PSUM"))
            zBpsum = ctx.enter_context(tc.tile_pool(name="zBpsum", bufs=2, space="PSUM"))
            spsum = ctx.enter_context(tc.tile_pool(name="spsum", bufs=2, space="PSUM"))
            rpsum = ctx.enter_context(tc.tile_pool(name="rpsum", bufs=2, space="PSUM"))

            expT_sb = const.tile([L, L], cdt)
            nc.sync.dma_start(out=expT_sb, in_=d_expT[:])
            expTT_sb = const.tile([L, L], cdt)
            nc.sync.dma_start(out=expTT_sb, in_=d_expTT[:])
            ones_col = const.tile([L, 1], cdt)
            nc.vector.memset(ones_col, 1.0)
            ones_row = const.tile([1, L], f32)
            nc.vector.memset(ones_row, 1.0)
            cA_sb = const.tile([1, BS], f32)
            nc.vector.memset(cA_sb, 0.0)
            cB_sb = const.tile([1, BS], f32)
            nc.vector.memset(cB_sb, 0.0)

            def _run_chain():
                pA = pApool.tile([L, BS], cdt, tag="pA")
                nc.sync.dma_start(out=pA, in_=d_p0[:])
                pB = pBpool.tile([L, BS], cdt, tag="pB")
                nc.sync.dma_start(out=pB, in_=d_b0[:])

                # emission chunks: forward ascending from 0, backward
                # descending from nsteps-1; track loaded chunks lazily
                em_tiles = {}

                def em_slice(i, scaled_by=None):
                    ci = i // T_CHUNK
                    if ci not in em_tiles:
                        lo = ci * T_CHUNK
                        hi = min(lo + T_CHUNK, nsteps)
                        tl = empool.tile([L, T_CHUNK, BS], cdt, tag=f"em{ci}")
                        nc.sync.dma_start(out=tl[:, : hi - lo, :], in_=d_E[:, lo:hi, :])
                        em_tiles[ci] = tl
                    sl = em_tiles[ci][:, i % T_CHUNK, :]
                    if scaled_by is not None:
                        em_scaled = sclp.tile([L, BS], cdt, tag="scl")
                        nc.vector.tensor_mul(em_scaled, sl, scaled_by)
                        sl = em_scaled
                    return sl

                def rescale(p_cur, c_sb, pend, t_apply):
                    s_ps = spsum.tile([1, BS], f32, tag="s")
                    nc.tensor.matmul(s_ps, lhsT=ones_col, rhs=p_cur,
                                     start=True, stop=True)
                    r_sb = smalls.tile([1, BS], f32, tag="r")
                    nc.vector.reciprocal(r_sb, s_ps)
                    rbc = rpsum.tile([L, BS], f32, tag="rbc")
                    nc.tensor.matmul(rbc, lhsT=ones_row, rhs=r_sb,
                                     start=True, stop=True)
                    ln_s = smalls.tile([1, BS], f32, tag="lns")
                    nc.scalar.activation(ln_s, s_ps, Ln)
                    nc.vector.tensor_add(c_sb, c_sb, ln_s)
                    pend[t_apply] = rbc

                pendA, pendB = {}, {}
                for k in range(max(F, G)):
                    # ---- forward step t = k+1 consumes em index k ----
                    if k < F:
                        tf = k + 1
                        rbcA = pendA.pop(tf, None)
                        em_f = em_slice(k, scaled_by=rbcA)
                        zA = zApsum.tile([L, BS], f32, tag="zA")
                        nc.tensor.matmul(zA, lhsT=expT_sb, rhs=pA,
                                         start=True, stop=True)
                        pA_new = pApool.tile([L, BS], cdt, tag="pA")
                        nc.vector.tensor_mul(pA_new, zA, em_f)
                        pA = pA_new
                        if tf % K_RESCALE == 0 and tf + APPLY_DELTA <= F:
                            rescale(pA, cA_sb, pendA, tf + APPLY_DELTA)
                    # ---- backward step consumes em index nsteps-1-k ----
                    if k < G:
                        ib = nsteps - 1 - k
                        tb = k + 1
                        rbcB = pendB.pop(tb, None)
                        em_b = em_slice(ib, scaled_by=rbcB)
                        qB = pBpool.tile([L, BS], cdt, tag="pB")
                        nc.vector.tensor_mul(qB, pB, em_b)
                        zB = zBpsum.tile([L, BS], f32, tag="zB")
                        nc.tensor.matmul(zB, lhsT=expTT_sb, rhs=qB,
                                         start=True, stop=True)
                        # copy PSUM->SBUF fused into next slot multiply:
                        # next iteration's tensor_mul reads zB (PSUM) directly
                        pB = zB
                        if (tb + K_RESCALE // 2) % K_RESCALE == 0 and tb + APPLY_DELTA <= G:
                            # pB is PSUM here; colsum matmul rhs must be SBUF,
                            # so rescale measures qB (pre-matmul state) instead
                            rescale(qB, cB_sb, pendB, tb + APPLY_DELTA)

                # ---- combine: logZ = ln(sum_l pA * pB_sbuf) + cA + cB ----
                # pB is PSUM [l, b]; multiply into SBUF with pA
                u = sclp.tile([L, BS], cdt, tag="u")
                nc.vector.tensor_mul(u, pB, pA)
                s_fin = spsum.tile([1, BS], f32, tag="s")
                nc.tensor.matmul(s_fin, lhsT=ones_col, rhs=u, start=True, stop=True)
                ln_fin = smalls.tile([1, BS], f32, tag="lns")
                nc.scalar.activation(ln_fin, s_fin, Ln)
                outv = smalls.tile([1, BS], f32, tag="outv")
                nc.vector.tensor_add(outv, ln_fin, cA_sb)
                nc.vector.tensor_add(outv, outv, cB_sb)
                nc.sync.dma_start(out=d_out[:], in_=outv)

            if repeat == 1:
                _run_chain()
            else:
                with tc.For_i(0, repeat, 1):
                    _run_chain()

    nc.compile()
    return nc


def _get_program(nsteps, repeat=1, use_bf16=None):
    if use_bf16 is None:
        use_bf16 = USE_BF16
    key = (nsteps, repeat, use_bf16)
    if key not in _BUILT:
        _BUILT[key] = _build(nsteps, repeat, use_bf16)
    return _BUILT[key]


def _prepare_in_maps(emissions, transitions, start_transitions, end_transitions,
                     nsteps=S - 1, use_bf16=None):
    """Host preprocessing -> (in_maps for 8 cores, c0[B])."""
    if use_bf16 is None:
        use_bf16 = USE_BF16
    if use_bf16:
        import ml_dtypes
        cdt = ml_dtypes.bfloat16
    else:
        cdt = np.float32
    expT = np.exp(transitions, dtype=np.float32).astype(cdt)      # [l, l']
    expEnd = np.exp(end_transitions, dtype=np.float32).reshape(L, 1).astype(cdt)

    alpha0 = start_transitions[None, :] + emissions[:, 0, :]      # [B, L] f32
    c0 = alpha0.max(axis=1)                                        # [B]
    p0_all = np.exp(alpha0 - c0[:, None]).T.astype(np.float32)     # [l, B]

    in_maps = []
    for k in range(NCORES):
        bs = slice(k * BS, (k + 1) * BS)
        # [l, t, b] = exp(em - D) transposed; steps 1..nsteps
        Ek = np.exp(
            np.ascontiguousarray(
                np.transpose(emissions[bs, 1 : nsteps + 1, :], (2, 1, 0))
            )
            - np.float32(D_SHIFT),
            dtype=np.float32,
        ).astype(cdt)
        in_maps.append(
            {
                "expT": expT,
                "expEnd": expEnd,
                "p0": np.ascontiguousarray(p0_all[:, bs]).astype(cdt),
                "emis": Ek,
            }
        )
    return in_maps, c0


def _forward_device(emissions, transitions, start_transitions, end_transitions,
                    nsteps=S - 1):
    """Run the device recursion; returns logZ [B] float64."""
    from concourse.bass_utils import run_bass_kernel_spmd

    global LAST_EXEC_TIME_NS, LAST_TRACE_PATH

    in_maps, c0 = _prepare_in_maps(
        emissions, transitions, start_transitions, end_transitions, nsteps
    )
    nc = _get_program(nsteps)
    trace = os.environ.get("CRF_TRACE", "") == "1"
    res = run_bass_kernel_spmd(
        nc, in_maps, core_ids=list(range(NCORES)), trace=trace
    )
    LAST_EXEC_TIME_NS = res.exec_time_ns
    if res.instructions_and_trace is not None:
        LAST_TRACE_PATH = res.instructions_and_trace[1]

    out = np.concatenate([res.results[k]["out"][0] for k in range(NCORES)])
    return out.astype(np.float64) + c0.astype(np.float64) + D_SHIFT * nsteps




def _prepare_in_maps_fb(emissions, transitions, start_transitions,
                        end_transitions, nsteps=S - 1):
    """Host preprocessing for the forward-backward program."""
    import ml_dtypes
    cdt = ml_dtypes.bfloat16
    expT = np.exp(transitions, dtype=np.float32)
    expTT = np.ascontiguousarray(expT.T)
    b0 = np.repeat(
        np.exp(end_transitions, dtype=np.float32).reshape(L, 1), BS, axis=1
    )

    alpha0 = start_transitions[None, :] + emissions[:, 0, :]
    c0 = alpha0.max(axis=1)
    p0_all = np.exp(alpha0 - c0[:, None]).T.astype(np.float32)

    in_maps = []
    for k in range(NCORES):
        bs = slice(k * BS, (k + 1) * BS)
        Ek = np.exp(
            np.ascontiguousarray(
                np.transpose(emissions[bs, 1 : nsteps + 1, :], (2, 1, 0))
            )
            - np.float32(D_SHIFT),
            dtype=np.float32,
        ).astype(cdt)
        in_maps.append(
            {
                "expT": expT.astype(cdt),
                "expTT": expTT.astype(cdt),
                "p0": np.ascontiguousarray(p0_all[:, bs]).astype(cdt),
                "b0": b0.astype(cdt),
                "emis": Ek,
            }
        )
    return in_maps, c0


def _forward_device_fb(emissions, transitions, start_transitions,
                       end_transitions, nsteps=S - 1):
    """Run the forward-backward device program; returns logZ [B] float64."""
    from concourse.bass_utils import run_bass_kernel_spmd

    in_maps, c0 = _prepare_in_maps_fb(
        emissions, transitions, start_transitions, end_transitions, nsteps
    )
    key = ("fb", nsteps)
    if key not in _BUILT:
        _BUILT[key] = _build_fb(nsteps)
    nc = _BUILT[key]
    res = run_bass_kernel_spmd(nc, in_maps, core_ids=list(range(NCORES)))
    out = np.concatenate([res.results[k]["out"][0] for k in range(NCORES)])
    return out.astype(np.float64) + c0.astype(np.float64) + D_SHIFT * nsteps




def _build_fb2(nsteps, repeat=1):
    """Paired forward-backward builder (bf16). Slot k runs the forward step
    k+1 (matmul expT then emission-multiply) and the backward step (matmul
    expT on the em-premultiplied state, then premultiply the NEXT backward
    emission). Both chains' DVE multiplies are fused into ONE tensor_mul on
    a [L, 2*BS] PSUM pair tile; emissions packed [l, slot, 2, b] on host
    (ones in unused half-slots). Host folds em_{S-1} into the backward
    initial state. Halves the serial chain: G = ceil(nsteps/2) slots."""
    import concourse.bacc as bacc
    import concourse.tile as tile
    from concourse import mybir

    f32 = mybir.dt.float32
    cdt = mybir.dt.bfloat16
    Ln = mybir.ActivationFunctionType.Ln
    G = (nsteps + 1) // 2    # slots / backward matmuls (1024 for 2047)
    F = nsteps - G           # forward matmuls (1023 for 2047)
    W = 2 * BS

    nc = bacc.Bacc(debug=False, name="crf_fb2")
    with tile.TileContext(nc) as tc:
        with ExitStack() as ctx:
            d_expT = nc.dram_tensor("expT", [L, L], cdt, kind="ExternalInput")
            d_expTT = nc.dram_tensor("expTT", [L, L], cdt, kind="ExternalInput")
            d_pq0 = nc.dram_tensor("pq0", [L, W], cdt, kind="ExternalInput")
            d_E = nc.dram_tensor("emis", [L, G, W], cdt, kind="ExternalInput")
            d_out = nc.dram_tensor("out", [1, BS], f32, kind="ExternalOutput")

            const = ctx.enter_context(tc.tile_pool(name="const", bufs=1))
            empool = ctx.enter_context(tc.tile_pool(name="empool", bufs=1))
            ppool = ctx.enter_context(tc.tile_pool(name="ppool", bufs=4))
            sclp = ctx.enter_context(tc.tile_pool(name="sclp", bufs=2))
            smalls = ctx.enter_context(tc.tile_pool(name="smalls", bufs=4))
            zpsum = ctx.enter_context(
                tc.tile_pool(name="zpsum", bufs=4, space="PSUM"))
            spsum = ctx.enter_context(
                tc.tile_pool(name="spsum", bufs=2, space="PSUM"))
            rpsum = ctx.enter_context(
                tc.tile_pool(name="rpsum", bufs=2, space="PSUM"))

            expT_sb = const.tile([L, L], cdt)
            nc.sync.dma_start(out=expT_sb, in_=d_expT[:])
            expTT_sb = const.tile([L, L], cdt)
            nc.sync.dma_start(out=expTT_sb, in_=d_expTT[:])
            ones_col = const.tile([L, 1], cdt)
            nc.vector.memset(ones_col, 1.0)
            ones_row = const.tile([1, L], f32)
            nc.vector.memset(ones_row, 1.0)
            c_sb = const.tile([1, BS], f32)
            nc.vector.memset(c_sb, 0.0)

            def _run_chain():
                pq = ppool.tile([L, W], cdt, tag="pq")
                nc.sync.dma_start(out=pq, in_=d_pq0[:])

                pendA, pendB = {}, {}
                em_tiles = {}

                def em_slice(k):
                    ci = k // T_CHUNK
                    if ci not in em_tiles:
                        lo = ci * T_CHUNK
                        hi = min(lo + T_CHUNK, G)
                        tl = empool.tile([L, T_CHUNK, W], cdt, tag=f"em{ci}")
                        nc.sync.dma_start(out=tl[:, : hi - lo, :],
                                          in_=d_E[:, lo:hi, :])
                        em_tiles[ci] = tl
                    return em_tiles[ci][:, k % T_CHUNK, :]

                def rescale(half_rhs, pend, t_apply):
                    s_ps = spsum.tile([1, BS], f32, tag="s")
                    nc.tensor.matmul(s_ps, lhsT=ones_col, rhs=half_rhs,
                                     start=True, stop=True)
                    r_sb = smalls.tile([1, BS], f32, tag="r")
                    nc.vector.reciprocal(r_sb, s_ps)
                    rbc = rpsum.tile([L, BS], f32, tag="rbc")
                    nc.tensor.matmul(rbc, lhsT=ones_row, rhs=r_sb,
                                     start=True, stop=True)
                    ln_s = smalls.tile([1, BS], f32, tag="lns")
                    nc.scalar.activation(ln_s, s_ps, Ln)
                    nc.vector.tensor_add(c_sb, c_sb, ln_s)
                    pend[t_apply] = rbc

                for k in range(G):
                    em_sl = em_slice(k)
                    rbcA = pendA.pop(k, None)
                    rbcB = pendB.pop(k, None)
                    if rbcA is not None or rbcB is not None:
                        em_scaled = sclp.tile([L, W], cdt, tag="scl")
                        if rbcA is not None:
                            nc.vector.tensor_mul(
                                em_scaled[:, :BS], em_sl[:, :BS], rbcA)
                        else:
                            nc.vector.tensor_copy(
                                out=em_scaled[:, :BS], in_=em_sl[:, :BS])
                        if rbcB is not None:
                            nc.vector.tensor_mul(
                                em_scaled[:, BS:], em_sl[:, BS:], rbcB)
                        else:
                            nc.vector.tensor_copy(
                                out=em_scaled[:, BS:], in_=em_sl[:, BS:])
                        em_sl = em_scaled

                    zz = zpsum.tile([L, W], f32, tag="zz")
                    if k < F:
                        nc.tensor.matmul(zz[:, :BS], lhsT=expT_sb,
                                         rhs=pq[:, :BS], start=True, stop=True)
                    else:
                        nc.vector.tensor_copy(out=zz[:, :BS], in_=pq[:, :BS])
                    nc.tensor.matmul(zz[:, BS:], lhsT=expTT_sb,
                                     rhs=pq[:, BS:], start=True, stop=True)
                    pq_new = ppool.tile([L, W], cdt, tag="pq")
                    nc.vector.tensor_mul(pq_new, zz, em_sl)
                    pq = pq_new

                    kk = k + 1
                    if kk % K_RESCALE == 0 and kk + APPLY_DELTA < F:
                        rescale(pq[:, :BS], pendA, kk + APPLY_DELTA)
                    if (kk + K_RESCALE // 2) % K_RESCALE == 0 \
                            and kk + APPLY_DELTA < G:
                        rescale(pq[:, BS:], pendB, kk + APPLY_DELTA)

                # combine: left = alpha_F (normalized), right = beta_F
                u = sclp.tile([L, BS], cdt, tag="u")
                nc.vector.tensor_mul(u, pq[:, :BS], pq[:, BS:])
                s_fin = spsum.tile([1, BS], f32, tag="s")
                nc.tensor.matmul(s_fin, lhsT=ones_col, rhs=u,
                                 start=True, stop=True)
                ln_fin = smalls.tile([1, BS], f32, tag="lns")
                nc.scalar.activation(ln_fin, s_fin, Ln)
                outv = smalls.tile([1, BS], f32, tag="outv")
                nc.vector.tensor_add(outv, ln_fin, c_sb)
                nc.sync.dma_start(out=d_out[:], in_=outv)

            if repeat == 1:
                _run_chain()
            else:
                with tc.For_i(0, repeat, 1):
                    _run_chain()

    nc.compile()
    return nc


def _prepare_in_maps_fb2(emissions, transitions, start_transitions,
                         end_transitions, nsteps=S - 1):
    """Host packing for the paired program.

    Et[:, i, :] = exp(em_{i+1} - D) for i = 0..nsteps-1 (transposed [l, b]).
    Forward half of slot k:  Et[:, k]            (k < F), ones otherwise.
    Backward half of slot k: Et[:, nsteps-2-k]   (k <= G-2), ones at G-1.
    pq0 = [p0 | Et[:, nsteps-1] * expEnd]  (first backward emission folded).
    """
    import ml_dtypes
    cdt = ml_dtypes.bfloat16
    G = (nsteps + 1) // 2
    F = nsteps - G
    expT = np.exp(transitions, dtype=np.float32)
    expTT = np.ascontiguousarray(expT.T)

    alpha0 = start_transitions[None, :] + emissions[:, 0, :]
    c0 = alpha0.max(axis=1)
    p0_all = np.exp(alpha0 - c0[:, None]).T.astype(np.float32)   # [l, B]
    b0 = np.exp(end_transitions, dtype=np.float32).reshape(L, 1)

    in_maps = []
    for kc in range(NCORES):
        bs = slice(kc * BS, (kc + 1) * BS)
        Et = np.exp(
            np.ascontiguousarray(
                np.transpose(emissions[bs, 1 : nsteps + 1, :], (2, 1, 0))
            )
            - np.float32(D_SHIFT),
            dtype=np.float32,
        )                                                         # [l, n, b]
        Ep = np.empty((L, G, 2, BS), dtype=np.float32)
        Ep[:, :F, 0, :] = Et[:, :F, :]
        if G > F:
            Ep[:, F:, 0, :] = 1.0
        Ep[:, : G - 1, 1, :] = Et[:, nsteps - 2 : nsteps - 2 - (G - 1) : -1, :]
        Ep[:, G - 1, 1, :] = 1.0
        pq0 = np.concatenate(
            [p0_all[:, bs], Et[:, nsteps - 1, :] * b0], axis=1
        )
        in_maps.append(
            {
                "expT": expT.astype(cdt),
                "expTT": expTT.astype(cdt),
                "pq0": np.ascontiguousarray(pq0).astype(cdt),
                "emis": np.ascontiguousarray(
                    Ep.reshape(L, G, 2 * BS)
                ).astype(cdt),
            }
        )
    return in_maps, c0


def _forward_device_fb2(emissions, transitions, start_transitions,
                        end_transitions, nsteps=S - 1):
    from concourse.bass_utils import run_bass_kernel_spmd

    in_maps, c0 = _prepare_in_maps_fb2(
        emissions, transitions, start_transitions, end_transitions, nsteps
    )
    key = ("fb2", nsteps)
    if key not in _BUILT:
        _BUILT[key] = _build_fb2(nsteps)
    nc = _BUILT[key]
    res = run_bass_kernel_spmd(nc, in_maps, core_ids=list(range(NCORES)))
    out = np.concatenate([res.results[k]["out"][0] for k in range(NCORES)])
    return out.astype(np.float64) + c0.astype(np.float64) + D_SHIFT * nsteps


def _score_host(emissions, mask, tags, transitions, start_transitions,
                end_transitions):
    """Gold path score, matching reference._crf_nll's gather path. float64."""
    em = emissions.astype(np.float64)
    T = transitions.astype(np.float64)
    startT = start_transitions.astype(np.float64)
    endT = end_transitions.astype(np.float64)

    valid = tags != IGNORE
    tags_safe = np.where(valid, tags, 0).astype(np.int64)
    vf = valid.astype(np.float64)

    score = startT[tags_safe[:, 0]] * vf[:, 0]
    prev_t = tags_safe[:, :-1]
    curr_t = tags_safe[:, 1:]
    trans_sc = T[prev_t, curr_t]
    em_sc = np.take_along_axis(em[:, 1:, :], curr_t[:, :, None], axis=2)[..., 0]
    score = score + np.sum((trans_sc + em_sc) * vf[:, 1:], axis=1)

    pos = np.arange(tags.shape[1])
    last_idx = np.max(np.where(valid, pos[None, :], -1), axis=1)
    last_tag = tags_safe[np.arange(tags.shape[0]), np.clip(last_idx, 0, S - 1)]
    score = score + np.where(last_idx >= 0, endT[last_tag], 0.0)
    return score


def _forward_numpy(emissions, mask, transitions, start_transitions,
                   end_transitions):
    """Fallback exact forward recursion (used only if mask isn't all ones)."""
    em = emissions.astype(np.float64)
    T = transitions.astype(np.float64)
    alpha = start_transitions.astype(np.float64)[None, :] + em[:, 0, :]
    for t in range(1, em.shape[1]):
        m = alpha.max(axis=1, keepdims=True)
        new = m + np.log(np.exp(alpha - m) @ np.exp(T)) + em[:, t, :]
        alpha = np.where(mask[:, t][:, None], new, alpha)
    m = alpha.max(axis=1, keepdims=True)
    return (
        m[:, 0]
        + np.log(
            np.exp(alpha - m) @ np.exp(end_transitions.astype(np.float64))
        )
    )


def kernel(emissions, mask, tags, transitions, start_transitions,
           end_transitions):
    emissions = np.asarray(emissions, dtype=np.float32)
    mask = np.asarray(mask)
    tags = np.asarray(tags)
    transitions = np.asarray(transitions, dtype=np.float32)
    start_transitions = np.asarray(start_transitions, dtype=np.float32)
    end_transitions = np.asarray(end_transitions, dtype=np.float32)

    if bool(mask.all()):
        logz = _forward_device_fb2(
            emissions, transitions, start_transitions, end_transitions
        )
    else:
        logz = _forward_numpy(
            emissions, mask, transitions, start_transitions, end_transitions
        )

    score = _score_host(
        emissions, mask, tags, transitions, start_transitions, end_transitions
    )
    return np.asarray(np.mean(logz - score), dtype=np.float32)
